# revision 9
# baseline (speedup 1.0000x reference)
"""Distributed multi-head attention kernel for 8 TRN2 NeuronCores.

Problem: x [4, 2048, 1024] -> qkv proj -> 16-head attention (d=64)
         -> out proj + bias -> [4, 2048, 1024].

Sharding (no collectives needed): core i handles batch b = i//2 and
query-half half = i%2 (1024 query tokens). Each core computes K/V for
its batch's full 2048-token sequence (duplicated within the pair of
cores sharing a batch) and Q only for its own 1024 tokens. The host
rotates the token axis per core so the core's query tokens are always
tokens [0, 1024) of its input -- attention is permutation-invariant
over keys, so K/V token order does not matter.

Per-core pipeline (all matmuls on TensorE):
  phase 1: QKV projections in f32r (full fp32 inputs), outputs cast to
           bf16: Q^T [d, q], K^T [d, k] (head pairs packed on 128
           partitions), V [k, d] stored in 65-wide per-head blocks with
           a ones column appended (yields softmax denominators for free
           during the PV matmul).
  phase 2: per head: S^T = K @ Q^T (PSUM), exp via ScalarE (scale=1/8
           fused, no max subtraction -- scores are O(1) by
           construction), P^T bf16; U^T[65, q] = [V|1]^T-style PV
           accumulation; row 64 = softmax denominator. Normalize via
           DVE reciprocal + gpsimd partition_broadcast + DVE multiply.
  phase 3: final[q, o] = sum_h U_h^T.T @ W_o[h] + 1^T b (bias as a K=1
           matmul), bf16 operands, fp32 PSUM accumulation, DMA out.
"""

import numpy as np
import ml_dtypes

B = 4
N = 2048
DIM = 1024
HEADS = 16
DH = 64
NQ = 1024  # query tokens per core
NCORES = 8

_CACHE = {}
DEBUG = False


def _build_nc():
    from contextlib import ExitStack

    import concourse.mybir as mybir
    import concourse.tile as tile
    from concourse import bacc

    f32 = mybir.dt.float32
    bf16 = mybir.dt.bfloat16
    EXP = mybir.ActivationFunctionType.Exp

    nc = bacc.Bacc("TRN2", target_bir_lowering=False, debug=False,
                   num_devices=NCORES)

    xt_d = nc.dram_tensor("xt", [DIM, N], bf16, kind="ExternalInput")
    wqkv_d = nc.dram_tensor("wqkv", [DIM, 3 * DIM], bf16, kind="ExternalInput")
    wo_d = nc.dram_tensor("wo", [HEADS, DH, DIM], bf16, kind="ExternalInput")
    brow_d = nc.dram_tensor("brow", [1, DIM], bf16, kind="ExternalInput")
    out_d = nc.dram_tensor("out", [NQ, DIM], f32, kind="ExternalOutput")
    if DEBUG:
        dbg_qt = nc.dram_tensor("dbg_qt", [128, NQ], bf16, kind="ExternalOutput")
        dbg_kt = nc.dram_tensor("dbg_kt", [128, N], bf16, kind="ExternalOutput")
        dbg_v = nc.dram_tensor("dbg_v", [128, 8, 65], bf16, kind="ExternalOutput")
        dbg_es = nc.dram_tensor("dbg_es", [128, 4, 512], bf16, kind="ExternalOutput")
        dbg_rec = nc.dram_tensor("dbg_rec", [1, 2, 512], f32, kind="ExternalOutput")
        dbg_bc = nc.dram_tensor("dbg_bc", [64, 2, 512], f32, kind="ExternalOutput")
        dbg_u = nc.dram_tensor("dbg_u", [64, NQ], bf16, kind="ExternalOutput")
        dbg_u1 = nc.dram_tensor("dbg_u1", [64, NQ], bf16, kind="ExternalOutput")
        dbg_u1 = nc.dram_tensor("dbg_u1", [64, NQ], bf16, kind="ExternalOutput")

    # round-robin PSUM->SBUF copy over the two idle-ish engines
    _cp = [0]

    def copy(dst, src):
        if _cp[0] % 2 == 0:
            nc.vector.tensor_copy(dst, src)
        else:
            nc.scalar.copy(dst, src)
        _cp[0] += 1

    with tile.TileContext(nc) as tc, ExitStack() as top:
        u_pool = top.enter_context(tc.tile_pool(name="u", bufs=1))
        const_pool = top.enter_context(tc.tile_pool(name="const", bufs=1))
        mm_psum = top.enter_context(tc.tile_pool(name="mmps", bufs=2, space="PSUM"))
        sp_psum = top.enter_context(tc.tile_pool(name="spps", bufs=1, space="PSUM"))
        u_psum = top.enter_context(tc.tile_pool(name="ups", bufs=1, space="PSUM"))
        es_pool = top.enter_context(tc.tile_pool(name="es", bufs=3))
        rec_pool = top.enter_context(tc.tile_pool(name="rec", bufs=2))
        bc_pool = top.enter_context(tc.tile_pool(name="bc", bufs=2))

        brow_t = const_pool.tile([1, DIM], bf16, tag="brow", name="brow")
        nc.sync.dma_start(brow_t[:], brow_d.ap()[:])
        ones_t = const_pool.tile([1, 128], bf16, tag="ones", name="ones")
        nc.gpsimd.memset(ones_t[:], 1.0)
        ones_bc = const_pool.tile([128, 64], f32, tag="ones_bc", name="ones_bc")
        nc.gpsimd.memset(ones_bc[:], 1.0)

        U = [u_pool.tile([64, NQ], bf16, tag=f"u{h}", name=f"u{h}") for h in range(HEADS)]

        with ExitStack() as ph12:
            xt_pool = ph12.enter_context(tc.tile_pool(name="xt", bufs=1))
            w_pool = ph12.enter_context(tc.tile_pool(name="w", bufs=2))

            xt = [xt_pool.tile([128, N], bf16, tag=f"xt{i}", name=f"xt{i}") for i in range(8)]
            for i in range(8):
                nc.sync.dma_start(xt[i][:], xt_d.ap()[i * 128:(i + 1) * 128, :])

            for half in range(2):  # heads [0..8) then [8..16)
                with ExitStack() as hg:
                    qt_pool = hg.enter_context(tc.tile_pool(name=f"qt{half}", bufs=1))
                    kt_pool = hg.enter_context(tc.tile_pool(name=f"kt{half}", bufs=1))
                    v_pool = hg.enter_context(tc.tile_pool(name=f"v{half}", bufs=1))
                    QT = [qt_pool.tile([128, NQ], bf16, tag=f"q{m}", name=f"q{m}") for m in range(4)]
                    KT = [kt_pool.tile([128, N], bf16, tag=f"k{m}", name=f"k{m}") for m in range(4)]
                    VT = [v_pool.tile([128, 8, 65], bf16, tag=f"v{mk}", name=f"v{mk}")
                          for mk in range(16)]

                    # ---- projections for this half's 8 heads ----
                    for role, col0 in (("q", half * 512),
                                       ("k", DIM + half * 512),
                                       ("v", 2 * DIM + half * 512)):
                        wb = [w_pool.tile([128, 512], bf16, tag=f"w{fc}", name=f"w{fc}")
                              for fc in range(8)]
                        for fc in range(8):
                            nc.sync.dma_start(
                                wb[fc][:],
                                wqkv_d.ap()[fc * 128:(fc + 1) * 128, col0:col0 + 512])
                        if role in ("q", "k"):
                            ntok = NQ if role == "q" else N
                            dest_tiles = QT if role == "q" else KT
                            for m4 in range(4):
                                for t in range(ntok // 512):
                                    ps = mm_psum.tile([128, 512], f32, tag="mm", name="mm")
                                    for fc in range(8):
                                        nc.tensor.matmul(
                                            ps[:],
                                            wb[fc][:, m4 * 128:(m4 + 1) * 128],
                                            xt[fc][:, t * 512:(t + 1) * 512],
                                            start=(fc == 0), stop=(fc == 7))
                                    copy(dest_tiles[m4][:, t * 512:(t + 1) * 512], ps[:])
                        else:
                            for mk in range(16):
                                ps = mm_psum.tile([128, 512], f32, tag="mm", name="mm")
                                for fc in range(8):
                                    nc.tensor.matmul(
                                        ps[:],
                                        xt[fc][:, mk * 128:(mk + 1) * 128],
                                        wb[fc][:],
                                        start=(fc == 0), stop=(fc == 7))
                                copy(VT[mk][:, :, 0:64],
                                     ps[:].rearrange("p (h d) -> p h d", d=64))
                                nc.gpsimd.memset(VT[mk][:, :, 64:65], 1.0)

                    if DEBUG and half == 0:
                        nc.sync.dma_start(dbg_qt.ap()[:], QT[0][:])
                        nc.sync.dma_start(dbg_kt.ap()[:], KT[0][:])
                        nc.sync.dma_start(dbg_v.ap()[:], VT[0][:])

                    # ---- attention for this half's 8 heads ----
                    for hh in range(8):
                        h = half * 8 + hh
                        pair = hh // 2
                        hb = (hh % 2) * 64
                        Ups = u_psum.tile([65, 2, 512], f32, tag="up", name="up")
                        for kg in range(8):
                            sp = sp_psum.tile([128, 4, 512], f32, tag="sp", name="sp")
                            for j2 in range(2):
                                k = kg * 2 + j2
                                for qc in range(2):
                                    nc.tensor.matmul(
                                        sp[:, j2 * 2 + qc, :],
                                        KT[pair][hb:hb + 64, k * 128:(k + 1) * 128],
                                        QT[pair][hb:hb + 64, qc * 512:(qc + 1) * 512],
                                        start=True, stop=True)
                            es = es_pool.tile([128, 4, 512], bf16, tag="es", name="es")
                            nc.scalar.activation(es[:], sp[:], EXP, scale=0.125)
                            if DEBUG and h == 0 and kg == 0:
                                nc.sync.dma_start(dbg_es.ap()[:], es[:])
                            for j2 in range(2):
                                k = kg * 2 + j2
                                for qc in range(2):
                                    nc.tensor.matmul(
                                        Ups[:, qc, :],
                                        VT[k][:, hh, :],
                                        es[:, j2 * 2 + qc, :],
                                        start=(kg == 0 and j2 == 0),
                                        stop=(kg == 7 and j2 == 1))
                        rec = rec_pool.tile([65, 2, 512], f32, tag="rec", name="rec")
                        nc.vector.reciprocal(rec[64:65, :, :], Ups[64:65, :, :])
                        if DEBUG and h == 0:
                            nc.sync.dma_start(dbg_rec.ap()[:], rec[64:65, :, :])
                        for qc in range(2):
                            bc = mm_psum.tile([64, 512], f32, tag="mm", name="bc")
                            nc.tensor.matmul(
                                bc[:], ones_bc[64:65, :],
                                rec[64:65, qc, :], start=True, stop=True)
                            bc_sb = bc_pool.tile([64, 512], f32, tag="bc",
                                                 name="bc_sb")
                            nc.vector.tensor_copy(bc_sb[:], bc[:])
                            nc.vector.tensor_mul(
                                U[h][:, qc * 512:(qc + 1) * 512],
                                Ups[0:64, qc, :], bc_sb[:])

        if DEBUG:
            nc.sync.dma_start(dbg_u.ap()[:], U[0][:])
            nc.sync.dma_start(dbg_u1.ap()[:], U[1][:])
            nc.sync.dma_start(dbg_u1.ap()[:], U[1][:])

        # ---- output projection + bias ----
        with ExitStack() as p3:
            wo_pool = p3.enter_context(tc.tile_pool(name="wo", bufs=1))
            fin_pool = p3.enter_context(tc.tile_pool(name="fin", bufs=2))
            WO = [wo_pool.tile([64, DIM], bf16, tag=f"wo{h}", name=f"wo{h}") for h in range(HEADS)]
            for h in range(HEADS):
                nc.sync.dma_start(WO[h][:], wo_d.ap()[h])
            for qf in range(8):
                st = fin_pool.tile([128, DIM], f32, tag="fin", name="fin")
                for of in range(2):
                    ps = mm_psum.tile([128, 512], f32, tag="mm", name="mm")
                    for h in range(HEADS):
                        nc.tensor.matmul(
                            ps[:],
                            U[h][:, qf * 128:(qf + 1) * 128],
                            WO[h][:, of * 512:(of + 1) * 512],
                            start=(h == 0), stop=False)
                    nc.tensor.matmul(
                        ps[:], ones_t[:, 0:128],
                        brow_t[:, of * 512:(of + 1) * 512],
                        start=False, stop=True)
                    copy(st[:, of * 512:(of + 1) * 512], ps[:])
                nc.sync.dma_start(
                    out_d.ap()[qf * 128:(qf + 1) * 128, :], st[:])

    nc.compile()
    return nc


def _get_nc():
    if "nc" not in _CACHE:
        _CACHE["nc"] = _build_nc()
    return _CACHE["nc"]


def _make_in_maps(x, w_qkv, w_out, b_out):
    bf = ml_dtypes.bfloat16
    wo = np.ascontiguousarray(w_out.reshape(HEADS, DH, DIM)).astype(bf)
    brow = np.asarray(b_out, np.float32).reshape(1, DIM).astype(bf)
    wqkv = np.ascontiguousarray(w_qkv, np.float32).astype(bf)
    in_maps = []
    for i in range(NCORES):
        b, half = i // 2, i % 2
        xt = np.asarray(x[b], np.float32).T.astype(bf)  # [DIM, N]
        if half:
            xt = np.concatenate([xt[:, NQ:], xt[:, :NQ]], axis=1)
        in_maps.append({
            "xt": np.ascontiguousarray(xt),
            "wqkv": wqkv,
            "wo": wo,
            "brow": brow,
        })
    return in_maps


def _assemble(results):
    out = np.empty((B, N, DIM), np.float32)
    for i in range(NCORES):
        b, half = i // 2, i % 2
        out[b, half * NQ:(half + 1) * NQ, :] = results[i]["out"]
    return out


def run(x, w_qkv, w_out, b_out, trace=False):
    """Run the kernel; returns (output, BassKernelResults)."""
    from concourse.bass_utils import run_bass_kernel_spmd
    nc = _get_nc()
    in_maps = _make_in_maps(x, w_qkv, w_out, b_out)
    res = run_bass_kernel_spmd(nc, in_maps, core_ids=list(range(NCORES)),
                               trace=trace)
    return _assemble(res.results), res


def kernel(x, w_qkv, w_out, b_out):
    out, _ = run(x, w_qkv, w_out, b_out, trace=False)
    return out


# revision 11
# speedup vs baseline: 1.1894x; 1.1894x over previous
"""Distributed multi-head attention kernel for 8 TRN2 NeuronCores.

Problem: x [4, 2048, 1024] -> qkv proj -> 16-head attention (d=64)
         -> out proj + bias -> [4, 2048, 1024].

Sharding (no collectives needed): core i handles batch b = i//2 and
query-half half = i%2 (1024 query tokens). Each core computes K/V for
its batch's full 2048-token sequence (duplicated within the pair of
cores sharing a batch) and Q only for its own 1024 tokens. The host
rotates the token axis per core so the core's query tokens are always
tokens [0, 1024) of its input -- attention is permutation-invariant
over keys, so K/V token order does not matter.

Per-core pipeline (all matmuls on TensorE):
  phase 1: QKV projections in f32r (full fp32 inputs), outputs cast to
           bf16: Q^T [d, q], K^T [d, k] (head pairs packed on 128
           partitions), V [k, d] stored in 65-wide per-head blocks with
           a ones column appended (yields softmax denominators for free
           during the PV matmul).
  phase 2: per head: S^T = K @ Q^T (PSUM), exp via ScalarE (scale=1/8
           fused, no max subtraction -- scores are O(1) by
           construction), P^T bf16; U^T[65, q] = [V|1]^T-style PV
           accumulation; row 64 = softmax denominator. Normalize via
           DVE reciprocal + gpsimd partition_broadcast + DVE multiply.
  phase 3: final[q, o] = sum_h U_h^T.T @ W_o[h] + 1^T b (bias as a K=1
           matmul), bf16 operands, fp32 PSUM accumulation, DMA out.
"""

import numpy as np
import ml_dtypes

B = 4
N = 2048
DIM = 1024
HEADS = 16
DH = 64
NQ = 1024  # query tokens per core
NCORES = 8

_CACHE = {}
DEBUG = False


def _build_nc():
    from contextlib import ExitStack

    import concourse.mybir as mybir
    import concourse.tile as tile
    from concourse import bacc

    f32 = mybir.dt.float32
    bf16 = mybir.dt.bfloat16
    f16 = mybir.dt.float16
    EXP = mybir.ActivationFunctionType.Exp

    nc = bacc.Bacc("TRN2", target_bir_lowering=False, debug=False,
                   num_devices=NCORES)

    xt_d = nc.dram_tensor("xt", [DIM, N], bf16, kind="ExternalInput")
    wqkv_d = nc.dram_tensor("wqkv", [DIM, 3 * DIM], bf16, kind="ExternalInput")
    wo_d = nc.dram_tensor("wo", [HEADS, DH, DIM], bf16, kind="ExternalInput")
    brow_d = nc.dram_tensor("brow", [1, DIM], bf16, kind="ExternalInput")
    out_d = nc.dram_tensor("out", [NQ, DIM], f32, kind="ExternalOutput")
    if DEBUG:
        dbg_qt = nc.dram_tensor("dbg_qt", [128, NQ], bf16, kind="ExternalOutput")
        dbg_kt = nc.dram_tensor("dbg_kt", [128, N], bf16, kind="ExternalOutput")
        dbg_v = nc.dram_tensor("dbg_v", [128, 8, 65], bf16, kind="ExternalOutput")
        dbg_es = nc.dram_tensor("dbg_es", [128, 4, 512], bf16, kind="ExternalOutput")
        dbg_rec = nc.dram_tensor("dbg_rec", [1, 2, 512], f16, kind="ExternalOutput")
        dbg_bc = nc.dram_tensor("dbg_bc", [64, 2, 512], f32, kind="ExternalOutput")
        dbg_u = nc.dram_tensor("dbg_u", [64, NQ], bf16, kind="ExternalOutput")
        dbg_u1 = nc.dram_tensor("dbg_u1", [64, NQ], bf16, kind="ExternalOutput")
        dbg_u1 = nc.dram_tensor("dbg_u1", [64, NQ], bf16, kind="ExternalOutput")

    # round-robin PSUM->SBUF copy over the two idle-ish engines
    _cp = [0]

    def copy(dst, src):
        if _cp[0] % 2 == 0:
            nc.vector.tensor_copy(dst, src)
        else:
            nc.scalar.copy(dst, src)
        _cp[0] += 1

    with tile.TileContext(nc) as tc, ExitStack() as top:
        u_pool = top.enter_context(tc.tile_pool(name="u", bufs=1))
        const_pool = top.enter_context(tc.tile_pool(name="const", bufs=1))
        mm_psum = top.enter_context(tc.tile_pool(name="mmps", bufs=2, space="PSUM"))
        sp_psum = top.enter_context(tc.tile_pool(name="spps", bufs=2, space="PSUM"))
        u_psum = top.enter_context(tc.tile_pool(name="ups", bufs=1, space="PSUM"))
        es_pool = top.enter_context(tc.tile_pool(name="es", bufs=3))
        rec_pool = top.enter_context(tc.tile_pool(name="rec", bufs=4))
        bc_pool = top.enter_context(tc.tile_pool(name="bc", bufs=2))
        uraw_pool = top.enter_context(tc.tile_pool(name="uraw", bufs=1))

        brow_t = const_pool.tile([1, DIM], bf16, tag="brow", name="brow")
        nc.sync.dma_start(brow_t[:], brow_d.ap()[:])
        ones_t = const_pool.tile([1, 128], bf16, tag="ones", name="ones")
        nc.gpsimd.memset(ones_t[:], 1.0)
        ones_bc = const_pool.tile([128, 64], f16, tag="ones_bc", name="ones_bc")
        nc.gpsimd.memset(ones_bc[:], 1.0)

        U = [u_pool.tile([64, NQ], bf16, tag=f"u{h}", name=f"u{h}") for h in range(HEADS)]

        with ExitStack() as ph12:
            xt_pool = ph12.enter_context(tc.tile_pool(name="xt", bufs=1))
            w_pool = ph12.enter_context(tc.tile_pool(name="w", bufs=2))

            xt = [xt_pool.tile([128, N], bf16, tag=f"xt{i}", name=f"xt{i}") for i in range(8)]
            for i in range(8):
                nc.sync.dma_start(xt[i][:], xt_d.ap()[i * 128:(i + 1) * 128, :])

            for half in range(2):  # heads [0..8) then [8..16)
                with ExitStack() as hg:
                    qt_pool = hg.enter_context(tc.tile_pool(name=f"qt{half}", bufs=1))
                    kt_pool = hg.enter_context(tc.tile_pool(name=f"kt{half}", bufs=1))
                    v_pool = hg.enter_context(tc.tile_pool(name=f"v{half}", bufs=1))
                    QT = [qt_pool.tile([128, NQ], bf16, tag=f"q{m}", name=f"q{m}") for m in range(4)]
                    KT = [kt_pool.tile([128, N], bf16, tag=f"k{m}", name=f"k{m}") for m in range(4)]
                    VT = [v_pool.tile([128, 8, 65], bf16, tag=f"v{mk}", name=f"v{mk}")
                          for mk in range(16)]

                    # ---- projections for this half's 8 heads ----
                    for role, col0 in (("q", half * 512),
                                       ("k", DIM + half * 512),
                                       ("v", 2 * DIM + half * 512)):
                        wb = [w_pool.tile([128, 512], bf16, tag=f"w{fc}", name=f"w{fc}")
                              for fc in range(8)]
                        for fc in range(8):
                            nc.sync.dma_start(
                                wb[fc][:],
                                wqkv_d.ap()[fc * 128:(fc + 1) * 128, col0:col0 + 512])
                        if role in ("q", "k"):
                            ntok = NQ if role == "q" else N
                            dest_tiles = QT if role == "q" else KT
                            for m4 in range(4):
                                for t in range(ntok // 512):
                                    ps = mm_psum.tile([128, 512], f32, tag="mm", name="mm")
                                    for fc in range(8):
                                        nc.tensor.matmul(
                                            ps[:],
                                            wb[fc][:, m4 * 128:(m4 + 1) * 128],
                                            xt[fc][:, t * 512:(t + 1) * 512],
                                            start=(fc == 0), stop=(fc == 7))
                                    copy(dest_tiles[m4][:, t * 512:(t + 1) * 512], ps[:])
                        else:
                            for mk in range(16):
                                ps = mm_psum.tile([128, 512], f32, tag="mm", name="mm")
                                for fc in range(8):
                                    nc.tensor.matmul(
                                        ps[:],
                                        xt[fc][:, mk * 128:(mk + 1) * 128],
                                        wb[fc][:],
                                        start=(fc == 0), stop=(fc == 7))
                                copy(VT[mk][:, :, 0:64],
                                     ps[:].rearrange("p (h d) -> p h d", d=64))
                                nc.gpsimd.memset(VT[mk][:, :, 64:65], 1.0)

                    if DEBUG and half == 0:
                        nc.sync.dma_start(dbg_qt.ap()[:], QT[0][:])
                        nc.sync.dma_start(dbg_kt.ap()[:], KT[0][:])
                        nc.sync.dma_start(dbg_v.ap()[:], VT[0][:])

                    # ---- attention for this half's 8 heads ----
                    recs = []
                    uraws = []
                    for hh in range(8):
                        h = half * 8 + hh
                        pair = hh // 2
                        hb = (hh % 2) * 64
                        Ups = u_psum.tile([65, 2, 512], f32, tag="up", name="up")
                        for k in range(16):
                            sp = sp_psum.tile([128, 2, 512], f32, tag="sp", name="sp")
                            for qc in range(2):
                                nc.tensor.matmul(
                                    sp[:, qc, :],
                                    KT[pair][hb:hb + 64, k * 128:(k + 1) * 128],
                                    QT[pair][hb:hb + 64, qc * 512:(qc + 1) * 512],
                                    start=True, stop=True)
                            es = es_pool.tile([128, 2, 512], bf16, tag="es", name="es")
                            nc.scalar.activation(es[:], sp[:], EXP, scale=0.125)
                            if DEBUG and h == 0 and k < 2:
                                nc.sync.dma_start(
                                    dbg_es.ap()[:, 2 * k:2 * k + 2, :], es[:])
                            for qc in range(2):
                                nc.tensor.matmul(
                                    Ups[:, qc, :],
                                    VT[k][:, hh, :],
                                    es[:, qc, :],
                                    start=(k == 0), stop=(k == 15))
                        # decoupled tail: stash unnormalized U + 1/D, free PSUM
                        uraw = uraw_pool.tile([64, NQ], bf16, tag=f"uraw{hh}",
                                              name=f"uraw{hh}")
                        nc.vector.tensor_copy(
                            uraw[:].rearrange("p (a b) -> p a b", a=2),
                            Ups[0:64, :, :])
                        rec = rec_pool.tile([65, NQ], f16, tag="rec", name="rec")
                        with nc.allow_low_precision(reason="softmax denom recip fp16"):
                            nc.vector.reciprocal(
                                rec[64:65, :].rearrange("p (a b) -> p a b", a=2),
                                Ups[64:65, :, :])
                        if DEBUG and h == 0:
                            nc.sync.dma_start(dbg_rec.ap()[:],
                                              rec[64:65, :].rearrange("p (a b) -> p a b", a=2))
                        recs.append(rec)
                        uraws.append(uraw)

                    # batched normalize for this half (off the PE critical path)
                    for hh in range(8):
                        h = half * 8 + hh
                        for qc in range(2):
                            bc = mm_psum.tile([64, 512], f32, tag="mm", name="bc")
                            nc.tensor.matmul(
                                bc[:], ones_bc[64:65, :],
                                recs[hh][64:65, qc * 512:(qc + 1) * 512],
                                start=True, stop=True)
                            bc_sb = bc_pool.tile([64, 512], f32, tag="bc",
                                                 name="bc_sb")
                            nc.vector.tensor_copy(bc_sb[:], bc[:])
                            nc.vector.tensor_mul(
                                U[h][:, qc * 512:(qc + 1) * 512],
                                uraws[hh][:, qc * 512:(qc + 1) * 512], bc_sb[:])

        if DEBUG:
            nc.sync.dma_start(dbg_u.ap()[:], U[0][:])
            nc.sync.dma_start(dbg_u1.ap()[:], U[1][:])
            nc.sync.dma_start(dbg_u1.ap()[:], U[1][:])

        # ---- output projection + bias ----
        with ExitStack() as p3:
            wo_pool = p3.enter_context(tc.tile_pool(name="wo", bufs=1))
            fin_pool = p3.enter_context(tc.tile_pool(name="fin", bufs=2))
            WO = [wo_pool.tile([64, DIM], bf16, tag=f"wo{h}", name=f"wo{h}") for h in range(HEADS)]
            for h in range(HEADS):
                nc.sync.dma_start(WO[h][:], wo_d.ap()[h])
            for qf in range(8):
                st = fin_pool.tile([128, DIM], f32, tag="fin", name="fin")
                for of in range(2):
                    ps = mm_psum.tile([128, 512], f32, tag="mm", name="mm")
                    for h in range(HEADS):
                        nc.tensor.matmul(
                            ps[:],
                            U[h][:, qf * 128:(qf + 1) * 128],
                            WO[h][:, of * 512:(of + 1) * 512],
                            start=(h == 0), stop=False)
                    nc.tensor.matmul(
                        ps[:], ones_t[:, 0:128],
                        brow_t[:, of * 512:(of + 1) * 512],
                        start=False, stop=True)
                    copy(st[:, of * 512:(of + 1) * 512], ps[:])
                nc.sync.dma_start(
                    out_d.ap()[qf * 128:(qf + 1) * 128, :], st[:])

    nc.compile()
    return nc


def _get_nc():
    if "nc" not in _CACHE:
        _CACHE["nc"] = _build_nc()
    return _CACHE["nc"]


def _make_in_maps(x, w_qkv, w_out, b_out):
    bf = ml_dtypes.bfloat16
    wo = np.ascontiguousarray(w_out.reshape(HEADS, DH, DIM)).astype(bf)
    brow = np.asarray(b_out, np.float32).reshape(1, DIM).astype(bf)
    wqkv = np.ascontiguousarray(w_qkv, np.float32).astype(bf)
    in_maps = []
    for i in range(NCORES):
        b, half = i // 2, i % 2
        xt = np.asarray(x[b], np.float32).T.astype(bf)  # [DIM, N]
        if half:
            xt = np.concatenate([xt[:, NQ:], xt[:, :NQ]], axis=1)
        in_maps.append({
            "xt": np.ascontiguousarray(xt),
            "wqkv": wqkv,
            "wo": wo,
            "brow": brow,
        })
    return in_maps


def _assemble(results):
    out = np.empty((B, N, DIM), np.float32)
    for i in range(NCORES):
        b, half = i // 2, i % 2
        out[b, half * NQ:(half + 1) * NQ, :] = results[i]["out"]
    return out


def run(x, w_qkv, w_out, b_out, trace=False):
    """Run the kernel; returns (output, BassKernelResults)."""
    from concourse.bass_utils import run_bass_kernel_spmd
    nc = _get_nc()
    in_maps = _make_in_maps(x, w_qkv, w_out, b_out)
    res = run_bass_kernel_spmd(nc, in_maps, core_ids=list(range(NCORES)),
                               trace=trace)
    return _assemble(res.results), res


def kernel(x, w_qkv, w_out, b_out):
    out, _ = run(x, w_qkv, w_out, b_out, trace=False)
    return out


# revision 17
# speedup vs baseline: 1.4887x; 1.2516x over previous
"""Distributed multi-head attention kernel for 8 TRN2 NeuronCores.

Problem: x [4, 2048, 1024] -> qkv proj -> 16-head attention (d=64)
         -> out proj + bias -> [4, 2048, 1024].

Sharding (no collectives): core i handles batch b = i//2 and query-half
half = i%2 (1024 query tokens). Each core computes K/V for its batch's
full 2048-token sequence (duplicated within the pair of cores sharing a
batch) and Q only for its own 1024 tokens. The host rotates the token
axis per core so the core's query tokens are always tokens [0, 1024) of
its input -- attention is permutation-invariant over keys, so K/V token
order does not matter.

Per-core pipeline (everything bf16 on the TensorE, fp32 PSUM accum):
  proj:  Q^T [d, q] / K^T [d, k] head-pairs packed on 128 partitions;
         V [k, d] in 65-wide per-head blocks with a ones column
         (the PV matmul then yields softmax denominators for free).
  attn:  per head: S^T = K @ Q^T -> exp on ScalarE (x0.125 fused, no
         max subtraction; scores are O(1) by construction) -> bf16 P^T
         -> PV accumulation U^T[65, q]; row 64 = denominator.
         Tail: U^T -> SBUF bf16 + 1/D (fp16) immediately (frees PSUM);
         normalize = K=1 ones matmul broadcast + DVE multiply, off the
         critical path.
  out:   two passes (heads 0-7 + bias, then heads 8-15) accumulating
         through a DRAM scratch so pass A fills the PE during the
         ACT-bound attention of the second half.

The two halves' projections and attention phases are arranged so the
PE always has matmul work while the ScalarE grinds through exp()
(keeps the PE HAM clock gate at 2.4 GHz).
"""

import numpy as np
import ml_dtypes

B = 4
N = 2048
DIM = 1024
HEADS = 16
DH = 64
NQ = 1024  # query tokens per core
NCORES = 8

_CACHE = {}


def _build_nc():
    from contextlib import ExitStack

    import concourse.bass as bass
    import concourse.mybir as mybir
    import concourse.tile as tile
    from concourse import bacc

    f32 = mybir.dt.float32
    bf16 = mybir.dt.bfloat16
    f16 = mybir.dt.float16
    EXP = mybir.ActivationFunctionType.Exp

    nc = bacc.Bacc("TRN2", target_bir_lowering=False, debug=False,
                   num_devices=NCORES)

    xt_d = nc.dram_tensor("xt", [DIM, N], bf16, kind="ExternalInput")
    wqkv_d = nc.dram_tensor("wqkv", [DIM, 3 * DIM], bf16, kind="ExternalInput")
    wo_d = nc.dram_tensor("wo", [HEADS, DH, DIM], bf16, kind="ExternalInput")
    brow_d = nc.dram_tensor("brow", [1, DIM], bf16, kind="ExternalInput")
    out_d = nc.dram_tensor("out", [NQ, DIM], f32, kind="ExternalOutput")

    with tile.TileContext(nc) as tc, ExitStack() as top:
        const_pool = top.enter_context(tc.tile_pool(name="const", bufs=1))
        mm_psum = top.enter_context(tc.tile_pool(name="mmps", bufs=2, space="PSUM"))
        sp_psum = top.enter_context(tc.tile_pool(name="spps", bufs=2, space="PSUM"))
        u_psum = top.enter_context(tc.tile_pool(name="ups", bufs=1, space="PSUM"))
        es_pool = top.enter_context(tc.tile_pool(name="es", bufs=3))
        rec_pool = top.enter_context(tc.tile_pool(name="rec", bufs=3))
        bc_pool = top.enter_context(tc.tile_pool(name="bc", bufs=2))
        uraw_a = top.enter_context(tc.tile_pool(name="uraw_a", bufs=1))
        dram_pool = top.enter_context(tc.tile_pool(name="dscr", bufs=1, space="DRAM"))

        brow_t = const_pool.tile([1, DIM], bf16, tag="brow", name="brow")
        nc.sync.dma_start(brow_t[:], brow_d.ap()[:])
        ones_t = const_pool.tile([1, 128], bf16, tag="ones", name="ones")
        nc.gpsimd.memset(ones_t[:], 1.0)
        ones_bc = const_pool.tile([128, 64], f16, tag="ones_bc", name="ones_bc")
        nc.gpsimd.memset(ones_bc[:], 1.0)

        uraw = [None] * HEADS

        def emit_proj(half, w_pool, xt, QT, KT, VT):
            """QKV projections for one half's 8 heads, in V, K, Q order."""
            for role, col0 in (("v", 2 * DIM + half * 512),
                               ("k", DIM + half * 512),
                               ("q", half * 512)):
                wb = [w_pool.tile([128, 512], bf16, tag=f"w{fc}", name=f"w{fc}")
                      for fc in range(8)]
                for fc in range(8):
                    nc.sync.dma_start(
                        wb[fc][:],
                        wqkv_d.ap()[fc * 128:(fc + 1) * 128, col0:col0 + 512])
                if role in ("q", "k"):
                    ntok = NQ if role == "q" else N
                    dest_tiles = QT if role == "q" else KT
                    for m4 in range(4):
                        for t in range(ntok // 512):
                            ps = mm_psum.tile([128, 512], f32, tag="mm", name="mm")
                            for fc in range(8):
                                nc.tensor.matmul(
                                    ps[:],
                                    wb[fc][:, m4 * 128:(m4 + 1) * 128],
                                    xt[fc][:, t * 512:(t + 1) * 512],
                                    start=(fc == 0), stop=(fc == 7))
                            nc.vector.tensor_copy(
                                dest_tiles[m4][:, t * 512:(t + 1) * 512], ps[:])
                else:
                    for mk in range(16):
                        ps = mm_psum.tile([128, 512], f32, tag="mm", name="mm")
                        for fc in range(8):
                            nc.tensor.matmul(
                                ps[:],
                                xt[fc][:, mk * 128:(mk + 1) * 128],
                                wb[fc][:],
                                start=(fc == 0), stop=(fc == 7))
                        nc.vector.tensor_copy(
                            VT[mk][:, :, 0:64],
                            ps[:].rearrange("p (h d) -> p h d", d=64))
                        nc.gpsimd.memset(VT[mk][:, :, 64:65], 1.0)

        def emit_attn(half, QT, KT, VT, uraw_pool):
            """Attention + per-head normalize for one half's 8 heads."""
            for hh in range(8):
                h = half * 8 + hh
                pair = hh // 2
                hb = (hh % 2) * 64
                Ups = u_psum.tile([65, 2, 512], f32, tag="up", name="up")
                for k in range(16):
                    sp = sp_psum.tile([128, 2, 512], f32, tag="sp", name="sp")
                    for qc in range(2):
                        nc.tensor.matmul(
                            sp[:, qc, :],
                            KT[pair][hb:hb + 64, k * 128:(k + 1) * 128],
                            QT[pair][hb:hb + 64, qc * 512:(qc + 1) * 512],
                            start=True, stop=True)
                    es = es_pool.tile([128, 2, 512], bf16, tag="es", name="es")
                    nc.scalar.activation(es[:], sp[:], EXP, scale=0.125)
                    for qc in range(2):
                        nc.tensor.matmul(
                            Ups[:, qc, :],
                            VT[k][:, hh, :],
                            es[:, qc, :],
                            start=(k == 0), stop=(k == 15))
                # free the PSUM slot fast: stash raw U + 1/D
                ur = uraw_pool.tile([64, NQ], bf16, tag=f"uraw{h}",
                                    name=f"uraw{h}")
                uraw[h] = ur
                nc.vector.tensor_copy(
                    ur[:].rearrange("p (a b) -> p a b", a=2), Ups[0:64, :, :])
                rec = rec_pool.tile([65, NQ], f16, tag="rec", name="rec")
                with nc.allow_low_precision(reason="softmax denom recip fp16"):
                    nc.vector.reciprocal(
                        rec[64:65, :].rearrange("p (a b) -> p a b", a=2),
                        Ups[64:65, :, :])
                # normalize (fills gaps; does not hold PSUM accumulation)
                for qc in range(2):
                    bc = mm_psum.tile([64, 512], f32, tag="mm", name="bc")
                    nc.tensor.matmul(
                        bc[:], ones_bc[64:65, :],
                        rec[64:65, qc * 512:(qc + 1) * 512],
                        start=True, stop=True)
                    bc_sb = bc_pool.tile([64, 512], f32, tag="bc", name="bc_sb")
                    nc.vector.tensor_copy(bc_sb[:], bc[:])
                    nc.vector.tensor_mul(
                        ur[:, qc * 512:(qc + 1) * 512],
                        ur[:, qc * 512:(qc + 1) * 512], bc_sb[:])

        # ---------------- emission ----------------
        with ExitStack() as xt_w:
            xt_pool = xt_w.enter_context(tc.tile_pool(name="xt", bufs=1))
            w_pool = xt_w.enter_context(tc.tile_pool(name="w", bufs=2))
            xt = [xt_pool.tile([128, N], bf16, tag=f"xt{i}", name=f"xt{i}")
                  for i in range(8)]
            for i in range(8):
                nc.sync.dma_start(xt[i][:], xt_d.ap()[i * 128:(i + 1) * 128, :])

            qkv0 = tc.alloc_tile_pool(name="qkv0", bufs=1)
            QT0 = [qkv0.tile([128, NQ], bf16, tag=f"q{m}", name=f"q0{m}")
                   for m in range(4)]
            KT0 = [qkv0.tile([128, N], bf16, tag=f"k{m}", name=f"k0{m}")
                   for m in range(4)]
            VT0 = [qkv0.tile([128, 8, 65], bf16, tag=f"v{mk}", name=f"v0{mk}")
                   for mk in range(16)]
            emit_proj(0, w_pool, xt, QT0, KT0, VT0)

            qkv1 = tc.alloc_tile_pool(name="qkv1", bufs=1, side="right")
            QT1 = [qkv1.tile([128, NQ], bf16, tag=f"q{m}", name=f"q1{m}")
                   for m in range(4)]
            KT1 = [qkv1.tile([128, N], bf16, tag=f"k{m}", name=f"k1{m}")
                   for m in range(4)]
            VT1 = [qkv1.tile([128, 8, 65], bf16, tag=f"v{mk}", name=f"v1{mk}")
                   for mk in range(16)]

            # attention(half 0) first in priority; proj(half 1) fills PE gaps
            emit_attn(0, QT0, KT0, VT0, uraw_a)
            emit_proj(1, w_pool, xt, QT1, KT1, VT1)

            qkv0.release()

            uraw_b = tc.alloc_tile_pool(name="uraw_b", bufs=1, side="right")
            wo_pool = tc.alloc_tile_pool(name="wo", bufs=1, side="right")
            st_pool = tc.alloc_tile_pool(name="st", bufs=2, side="right")
            WO = [wo_pool.tile([64, DIM], bf16, tag=f"wo{h}", name=f"wo{h}")
                  for h in range(HEADS)]
            for h in range(HEADS):
                nc.sync.dma_start(WO[h][:], wo_d.ap()[h])
            SCR = [dram_pool.tile([128, DIM], f32, tag=f"scr{qf}",
                                  name=f"scr{qf}") for qf in range(8)]

            emit_attn(1, QT1, KT1, VT1, uraw_b)

            # pass A: heads 0-7 + bias -> DRAM scratch (fills attn1 PE gaps)
            for qf in range(8):
                st = st_pool.tile([128, DIM], f32, tag="st", name="st")
                for of in range(2):
                    ps = mm_psum.tile([128, 512], f32, tag="mm", name="mm")
                    for hh in range(8):
                        nc.tensor.matmul(
                            ps[:],
                            uraw[hh][:, qf * 128:(qf + 1) * 128],
                            WO[hh][:, of * 512:(of + 1) * 512],
                            start=(hh == 0), stop=False)
                    nc.tensor.matmul(
                        ps[:], ones_t[:, 0:128],
                        brow_t[:, of * 512:(of + 1) * 512],
                        start=False, stop=True)
                    nc.vector.tensor_copy(st[:, of * 512:(of + 1) * 512], ps[:])
                nc.sync.dma_start(SCR[qf][:], st[:])

            # pass B: heads 8-15 + scratch -> out
            for qf in range(8):
                stb = st_pool.tile([128, DIM], f32, tag="stb", name="stb")
                nc.sync.dma_start(stb[:], SCR[qf][:])
                for of in range(2):
                    ps = mm_psum.tile([128, 512], f32, tag="mm", name="mm")
                    for hh in range(8):
                        nc.tensor.matmul(
                            ps[:],
                            uraw[8 + hh][:, qf * 128:(qf + 1) * 128],
                            WO[8 + hh][:, of * 512:(of + 1) * 512],
                            start=(hh == 0), stop=(hh == 7))
                    nc.vector.tensor_add(
                        stb[:, of * 512:(of + 1) * 512],
                        stb[:, of * 512:(of + 1) * 512], ps[:])
                nc.sync.dma_start(out_d.ap()[qf * 128:(qf + 1) * 128, :], stb[:])

            st_pool.release()
            wo_pool.release()
            uraw_b.release()
            qkv1.release()

    nc.compile()
    return nc


def _get_nc():
    if "nc" not in _CACHE:
        _CACHE["nc"] = _build_nc()
    return _CACHE["nc"]


def _make_in_maps(x, w_qkv, w_out, b_out):
    bf = ml_dtypes.bfloat16
    wo = np.ascontiguousarray(w_out.reshape(HEADS, DH, DIM)).astype(bf)
    brow = np.asarray(b_out, np.float32).reshape(1, DIM).astype(bf)
    wqkv = np.ascontiguousarray(w_qkv, np.float32).astype(bf)
    in_maps = []
    for i in range(NCORES):
        b, half = i // 2, i % 2
        xt = np.asarray(x[b], np.float32).T.astype(bf)  # [DIM, N]
        if half:
            xt = np.concatenate([xt[:, NQ:], xt[:, :NQ]], axis=1)
        in_maps.append({
            "xt": np.ascontiguousarray(xt),
            "wqkv": wqkv,
            "wo": wo,
            "brow": brow,
        })
    return in_maps


def _assemble(results):
    out = np.empty((B, N, DIM), np.float32)
    for i in range(NCORES):
        b, half = i // 2, i % 2
        out[b, half * NQ:(half + 1) * NQ, :] = results[i]["out"]
    return out


def run(x, w_qkv, w_out, b_out, trace=False):
    """Run the kernel; returns (output, BassKernelResults)."""
    from concourse.bass_utils import run_bass_kernel_spmd
    nc = _get_nc()
    in_maps = _make_in_maps(x, w_qkv, w_out, b_out)
    res = run_bass_kernel_spmd(nc, in_maps, core_ids=list(range(NCORES)),
                               trace=trace)
    return _assemble(res.results), res


def kernel(x, w_qkv, w_out, b_out):
    out, _ = run(x, w_qkv, w_out, b_out, trace=False)
    return out
